# revision 4
# baseline (speedup 1.0000x reference)
"""Trainium2 Bass kernel for nn_ContrastiveLoss2 (SimCLR-style NT-Xent loss).

Math (matches the jax reference):
    z  = concat([z_augment, z_orig])                       # [N=8192, D=256]
    zn = z / max(||z||, eps)                               # row L2 normalize
    S  = zn @ zn.T                                         # cosine sim [N, N]
    loss_i = -S[i, i+-B]/tau + log( sum_{j != i} exp(S[i,j]/tau) )
    out = mean_i loss_i                                    # tau = 0.5

Key identity: the softmax denominator is the full row sum of exp(S/tau)
minus the diagonal term exp(S_ii/tau).

Distribution: data-parallel over the 8192 rows -> 1024 rows per core.
Each core receives the full z ROTATED so that its own rows sit at
[0:1024) and the positive partners at [4096:5120).  Pure SPMD, no
collectives; the host sums the 8 per-core partial losses.

Per-core pipeline (engine assignment in parentheses):
  - load z in 8 groups of 1024 rows (SP DMA)
  - per-tile sum-of-squares via fused tensor_tensor_reduce (DVE)
  - 1/norm = exp(-0.5*ln(sumsq)) (ACT, one table set)
  - zn = z * invnorm -> bf16, one fused DVE op per group slice
  - bf16 bounce to DRAM (Pool DMA) + transposed reload (SP xbar DMA)
  - bf16 -> fp8e4 cast of the transposed operand (Pool)
  - S row-blocks via fp8 DoubleRow matmul (PE), K=256 in one pass
  - exp(2*S) + row-sum via activation accum_out (ACT), 2048-col chunks,
    column-major over the sim matrix so the ACT queue never starves
  - loss assembly (DVE/ACT) -> [128, 8] per-row losses -> DRAM

Groups 0/1 are prepped in fine 256-row slices to shorten the pipeline
ramp before the first exp; later groups are prepped 1024 rows at a
time, emitted between exp column-blocks so every engine queue stays
in dependency order.
"""

import sys

import numpy as np

try:
    import concourse  # noqa: F401
except ImportError:  # pragma: no cover
    sys.path.insert(0, "/opt/trn_rl_repo")

N_CORES = 8
N = 8192          # total rows (2B)
D = 256           # feature dim
B = 4096          # batch (positive offset)
ROWS_PER_CORE = N // N_CORES   # 1024
P = 128           # SBUF partitions
NT = N // P       # 64 natural row-tiles
NGRP = 8          # prep groups (1024 rows each)
TPG = NT // NGRP  # 8 tiles per group
NI = ROWS_PER_CORE // P        # 8 own row-tiles
CH = 512          # matmul chunk (one PSUM bank of fp32)
CC = 2048         # ACT exp chunk width = 4 PSUM banks
NCC = N // CC     # 4 column chunks
TAU = 0.5


def _kernel_body(ctx, tc, out_ap, zr_ap):
    import concourse.bass as bass  # noqa: F401
    from concourse import mybir

    nc = tc.nc
    f32 = mybir.dt.float32
    bf16 = mybir.dt.bfloat16
    fp8 = mybir.dt.float8e4
    Fn = mybir.ActivationFunctionType
    Op = mybir.AluOpType

    p_znat = ctx.enter_context(tc.tile_pool(name="znat", bufs=1))
    p_zn = ctx.enter_context(tc.tile_pool(name="zn", bufs=1))
    p_zntb = ctx.enter_context(tc.tile_pool(name="zntb", bufs=1))
    p_znt = ctx.enter_context(tc.tile_pool(name="znt", bufs=1))
    p_stats = ctx.enter_context(tc.tile_pool(name="stats", bufs=1))
    p_sq = ctx.enter_context(tc.tile_pool(name="sq", bufs=2))
    p_ex = ctx.enter_context(tc.tile_pool(name="ex", bufs=2))
    p_ps = ctx.enter_context(tc.tile_pool(name="ps", bufs=2, space="PSUM"))
    p_dram = ctx.enter_context(tc.tile_pool(name="dram", bufs=1, space="DRAM"))

    # per-group tiles: keeps the dependency tracker's byte ranges disjoint
    # so transposes/casts of group g never serialize behind group g+1
    znat = [p_znat.tile([P, TPG * D], f32, tag=f"znat{g}", name=f"znat{g}")
            for g in range(NGRP)]
    zn = [p_zn.tile([P, TPG * D], bf16, tag=f"zn{g}", name=f"zn{g}")
          for g in range(NGRP)]
    zntb = [p_zntb.tile([P, 2, TPG * P], bf16, tag=f"zntb{g}", name=f"zntb{g}")
            for g in range(NGRP)]
    znt = [p_znt.tile([P, 2, TPG * P], fp8, tag=f"znt{g}", name=f"znt{g}")
           for g in range(NGRP)]
    zbounce = [p_dram.tile([TPG * P, D], bf16, tag=f"zb{g}", name=f"zb{g}")
               for g in range(NGRP)]
    ss = p_stats.tile([P, NT], f32, tag="ss")     # per-row sum of squares
    inv = p_stats.tile([P, NT], f32, tag="inv")   # per-row 1/norm
    posr = p_stats.tile([P, NI], f32, tag="posr")  # raw dot(z_i, z_partner)
    sums = p_stats.tile([P, NI * NCC], f32, tag="sums")  # exp row-sum parts

    def prep(g, nsl=1):
        """Prepare group g (1024 rows) in nsl pipeline slices."""
        spt = TPG // nsl          # tiles per slice
        for s in range(nsl):
            t0 = g * TPG + s * spt        # global first tile of slice
            l0 = s * spt                  # tile offset within group tiles
            # ---- load rotated rows [t0*128, (t0+spt)*128) (SP) ----
            nc.sync.dma_start(
                out=znat[g][:, l0 * D:(l0 + spt) * D].rearrange(
                    "p (t c) -> p t c", c=D),
                in_=zr_ap[t0 * P:(t0 + spt) * P, :].rearrange(
                    "(t p) c -> p t c", p=P),
            )

            # ---- per-tile sumsq via fused mul+reduce (DVE) ----
            for t in range(spt):
                a = znat[g][:, (l0 + t) * D:(l0 + t + 1) * D]
                sq = p_sq.tile([P, D], f32, tag="sq", name="sq")
                nc.vector.tensor_tensor_reduce(
                    out=sq[:], in0=a, in1=a,
                    scale=1.0, scalar=0.0,
                    op0=Op.mult, op1=Op.add,
                    accum_out=ss[:, t0 + t:t0 + t + 1],
                )

            # ---- 1/norm = exp(-0.5 * ln(sumsq)); single ACT table set ----
            lns = p_sq.tile([P, spt], f32, tag="lns", name="lns")
            nc.scalar.activation(lns[:], ss[:, t0:t0 + spt], Fn.Ln)
            nc.scalar.activation(
                inv[:, t0:t0 + spt], lns[:], Fn.Exp, scale=-0.5)

            # ---- normalize slice to bf16 in ONE DVE op (broadcast inv) ----
            inv_bc = inv[:, t0:t0 + spt].rearrange(
                "p (t o) -> p t o", o=1).broadcast_to((P, spt, D))
            nc.vector.tensor_mul(
                zn[g][:, l0 * D:(l0 + spt) * D].rearrange(
                    "p (t c) -> p t c", c=D),
                znat[g][:, l0 * D:(l0 + spt) * D].rearrange(
                    "p (t c) -> p t c", c=D),
                inv_bc)

            # ---- bounce bf16 to DRAM (Pool), then 2 transposed xbar
            # ---- loads [rows,128]->[128,rows] (SP), then fp8 cast (Pool) ---
            nc.gpsimd.dma_start(
                out=zbounce[g][l0 * P:(l0 + spt) * P, :].rearrange(
                    "(t p) c -> p t c", p=P),
                in_=zn[g][:, l0 * D:(l0 + spt) * D].rearrange(
                    "p (t c) -> p t c", c=D),
            )
            for k in range(2):
                nc.sync.dma_start_transpose(
                    zntb[g][:, k, l0 * P:(l0 + spt) * P],
                    zbounce[g][l0 * P:(l0 + spt) * P, k * P:(k + 1) * P],
                )
            nc.gpsimd.tensor_copy(
                out=znt[g][:, :, l0 * P:(l0 + spt) * P],
                in_=zntb[g][:, :, l0 * P:(l0 + spt) * P])

        # ---- raw positive dots: own tiles (g0) x partner tiles (g4) ----
        if g == 4:
            for i in range(NI):
                sq = p_sq.tile([P, D], f32, tag="sq", name="sqp")
                nc.vector.tensor_tensor_reduce(
                    out=sq[:],
                    in0=znat[0][:, i * D:(i + 1) * D],
                    in1=znat[4][:, i * D:(i + 1) * D],
                    scale=1.0, scalar=0.0,
                    op0=Op.mult, op1=Op.add,
                    accum_out=posr[:, i:i + 1],
                )

    prep(0, nsl=4)
    prep(1, nsl=2)

    # ---- main loop: column-major S row-blocks + fused exp/rowsum ----
    for cc in range(NCC):
        for t in range(NI):
            ps = p_ps.tile([P, CC], f32, tag="ps", name="psmm")
            for jc in range(CC // CH):
                col = cc * CC + jc * CH
                g, c0 = divmod(col, TPG * P)
                nc.tensor.matmul(
                    ps[:, jc * CH:(jc + 1) * CH],
                    lhsT=znt[0][:, :, t * P:(t + 1) * P],
                    rhs=znt[g][:, :, c0:c0 + CH],
                    start=True, stop=True,
                    perf_mode=mybir.MatmulPerfMode.DoubleRow,
                )
            ex = p_ex.tile([P, CC], fp8, tag="ex", name="ex")
            nc.scalar.activation(
                ex[:], ps[:], Fn.Exp, scale=2.0,
                accum_out=sums[:, t * NCC + cc:t * NCC + cc + 1],
            )
        # prep the two groups needed by column block cc+1
        g = 2 * cc + 2
        if g < NGRP:
            prep(g)
            prep(g + 1)

    # ---- loss assembly: loss = -2*pos + ln(rowsum - exp(2*selfsim)) ----
    totals = p_stats.tile([P, NI], f32, tag="tot")
    nc.vector.tensor_reduce(
        totals[:], sums[:].rearrange("p (i c) -> p i c", c=NCC),
        axis=mybir.AxisListType.X, op=Op.add,
    )
    s1 = p_stats.tile([P, NI], f32, tag="s1")
    nc.vector.tensor_tensor(s1[:], ss[:, 0:NI], inv[:, 0:NI], op=Op.mult)
    s2 = p_stats.tile([P, NI], f32, tag="s2")
    nc.vector.tensor_tensor(s2[:], s1[:], inv[:, 0:NI], op=Op.mult)
    es = p_stats.tile([P, NI], f32, tag="es")
    nc.scalar.activation(es[:], s2[:], Fn.Exp, scale=2.0)
    neg = p_stats.tile([P, NI], f32, tag="neg")
    nc.vector.tensor_sub(neg[:], totals[:], es[:])
    lg = p_stats.tile([P, NI], f32, tag="lg")
    nc.scalar.activation(lg[:], neg[:], Fn.Ln)
    p1 = p_stats.tile([P, NI], f32, tag="p1")
    nc.vector.tensor_tensor(p1[:], posr[:], inv[:, 0:NI], op=Op.mult)
    p2 = p_stats.tile([P, NI], f32, tag="p2")
    nc.vector.tensor_tensor(
        p2[:], p1[:], inv[:, NT // 2:NT // 2 + NI], op=Op.mult)
    loss = p_stats.tile([P, NI], f32, tag="loss")
    nc.vector.scalar_tensor_tensor(
        out=loss[:], in0=p2[:], scalar=-2.0 / (2.0 * TAU), in1=lg[:],
        op0=Op.mult, op1=Op.add,
    )
    nc.sync.dma_start(out=out_ap, in_=loss[:])


def build_nc():
    """Build (once) the Bass module shared by all 8 cores."""
    from contextlib import ExitStack

    from concourse import bacc, mybir
    import concourse.tile as tile

    nc = bacc.Bacc("TRN2", target_bir_lowering=False, debug=False)
    zr = nc.dram_tensor("zr", [N, D], mybir.dt.float32,
                        kind="ExternalInput").ap()
    out = nc.dram_tensor("out", [P, NI], mybir.dt.float32,
                         kind="ExternalOutput").ap()
    with tile.TileContext(nc) as tc:
        with ExitStack() as ctx:
            _kernel_body(ctx, tc, out, zr)
    return nc


_NC = None


def _get_nc(finalized=True):
    global _NC
    if _NC is None:
        _NC = build_nc()
    if finalized and not _NC.is_finalized():
        _NC.finalize()
    return _NC


def make_in_maps(z_orig, z_augment):
    z = np.ascontiguousarray(
        np.concatenate([np.asarray(z_augment, dtype=np.float32),
                        np.asarray(z_orig, dtype=np.float32)], axis=0))
    return [{"zr": np.roll(z, -ROWS_PER_CORE * c, axis=0)}
            for c in range(N_CORES)]


def reduce_outputs(results):
    total = 0.0
    for r in results:
        total += float(np.asarray(r["out"], dtype=np.float64).sum())
    return np.float32(total / N)


def kernel(z_orig, z_augment):
    from concourse.bass_utils import run_bass_kernel_spmd

    nc = _get_nc()
    in_maps = make_in_maps(z_orig, z_augment)
    res = run_bass_kernel_spmd(nc, in_maps, core_ids=list(range(N_CORES)))
    return reduce_outputs(res.results)


# revision 6
# speedup vs baseline: 1.2150x; 1.2150x over previous
"""Trainium2 Bass kernel for nn_ContrastiveLoss2 (SimCLR-style NT-Xent loss).

Math (matches the jax reference):
    z  = concat([z_augment, z_orig])                       # [N=8192, D=256]
    zn = z / max(||z||, eps)                               # row L2 normalize
    S  = zn @ zn.T                                         # cosine sim [N, N]
    loss_i = -S[i, i+-B]/tau + log( sum_{j != i} exp(S[i,j]/tau) )
    out = mean_i loss_i                                    # tau = 0.5

Key identity: the softmax denominator is the full row sum of exp(S/tau)
minus the diagonal term exp(S_ii/tau).

Distribution: data-parallel over the 8192 rows -> 1024 rows per core.
Each core receives the full z ROTATED so that its own rows sit at
[0:1024) and the positive partners at [4096:5120).  Pure SPMD, no
collectives; the host sums the 8 per-core partial losses.

Per-core pipeline (engine assignment in parentheses):
  - load z in 8 groups of 1024 rows (SP DMA)
  - per-tile sum-of-squares via fused tensor_tensor_reduce (DVE)
  - 1/norm = exp(-0.5*ln(sumsq)) (ACT, one table set)
  - zn = z * invnorm -> bf16, one fused DVE op per group slice
  - bf16 bounce to DRAM (Pool DMA) + transposed reload (SP xbar DMA)
  - bf16 -> fp8e4 cast of the transposed operand (Pool)
  - S row-blocks via fp8 DoubleRow matmul (PE), K=256 in one pass
  - exp(2*S) + row-sum via activation accum_out (ACT), 2048-col chunks,
    column-major over the sim matrix so the ACT queue never starves
  - loss assembly (DVE/ACT) -> [128, 8] per-row losses -> DRAM

Groups 0/1 are prepped in fine 256-row slices to shorten the pipeline
ramp before the first exp; later groups are prepped 1024 rows at a
time, emitted between exp column-blocks so every engine queue stays
in dependency order.
"""

import sys

import numpy as np

try:
    import concourse  # noqa: F401
except ImportError:  # pragma: no cover
    sys.path.insert(0, "/opt/trn_rl_repo")

N_CORES = 8
N = 8192          # total rows (2B)
D = 256           # feature dim
B = 4096          # batch (positive offset)
ROWS_PER_CORE = N // N_CORES   # 1024
P = 128           # SBUF partitions
NT = N // P       # 64 natural row-tiles
NGRP = 8          # prep groups (1024 rows each)
TPG = NT // NGRP  # 8 tiles per group
NI = ROWS_PER_CORE // P        # 8 own row-tiles
CH = 512          # matmul chunk (one PSUM bank of fp32)
CC = 2048         # ACT exp chunk width = 4 PSUM banks
NCC = N // CC     # 4 column chunks
TAU = 0.5


def _kernel_body(ctx, tc, out_ap, zr_ap):
    import concourse.bass as bass  # noqa: F401
    from concourse import mybir

    nc = tc.nc
    f32 = mybir.dt.float32
    bf16 = mybir.dt.bfloat16
    fp8 = mybir.dt.float8e4
    Fn = mybir.ActivationFunctionType
    Op = mybir.AluOpType

    p_znat = ctx.enter_context(tc.tile_pool(name="znat", bufs=1))
    p_zn = ctx.enter_context(tc.tile_pool(name="zn", bufs=1))
    p_zntb = ctx.enter_context(tc.tile_pool(name="zntb", bufs=1))
    p_znt = ctx.enter_context(tc.tile_pool(name="znt", bufs=1))
    p_stats = ctx.enter_context(tc.tile_pool(name="stats", bufs=1))
    p_sq = ctx.enter_context(tc.tile_pool(name="sq", bufs=2))
    p_ex = ctx.enter_context(tc.tile_pool(name="ex", bufs=2))
    p_ps = ctx.enter_context(tc.tile_pool(name="ps", bufs=2, space="PSUM"))
    p_dram = ctx.enter_context(tc.tile_pool(name="dram", bufs=1, space="DRAM"))

    # per-group tiles: keeps the dependency tracker's byte ranges disjoint
    # so transposes/casts of group g never serialize behind group g+1
    znat = [p_znat.tile([P, TPG * D], f32, tag=f"znat{g}", name=f"znat{g}")
            for g in range(NGRP)]
    zn = [p_zn.tile([P, TPG * D], bf16, tag=f"zn{g}", name=f"zn{g}")
          for g in range(NGRP)]
    zntb = [p_zntb.tile([P, 2, TPG * P], bf16, tag=f"zntb{g}", name=f"zntb{g}")
            for g in range(NGRP)]
    znt = [p_znt.tile([P, 2, TPG * P], fp8, tag=f"znt{g}", name=f"znt{g}")
           for g in range(NGRP)]
    zbounce = [p_dram.tile([TPG * P, D], bf16, tag=f"zb{g}", name=f"zb{g}")
               for g in range(NGRP)]
    ss = p_stats.tile([P, NT], f32, tag="ss")     # per-row sum of squares
    inv = p_stats.tile([P, NT], f32, tag="inv")   # per-row 1/norm
    posr = p_stats.tile([P, NI], f32, tag="posr")  # raw dot(z_i, z_partner)
    sums = p_stats.tile([P, NI * NCC], f32, tag="sums")  # exp row-sum parts

    def prep(g, nsl=1):
        """Prepare group g (1024 rows) in nsl pipeline slices."""
        spt = TPG // nsl          # tiles per slice
        for s in range(nsl):
            t0 = g * TPG + s * spt        # global first tile of slice
            l0 = s * spt                  # tile offset within group tiles
            # ---- load rotated rows [t0*128, (t0+spt)*128) (SP) ----
            nc.sync.dma_start(
                out=znat[g][:, l0 * D:(l0 + spt) * D].rearrange(
                    "p (t c) -> p t c", c=D),
                in_=zr_ap[t0 * P:(t0 + spt) * P, :].rearrange(
                    "(t p) c -> p t c", p=P),
            )

            # ---- per-tile sumsq via fused mul+reduce (DVE) ----
            for t in range(spt):
                a = znat[g][:, (l0 + t) * D:(l0 + t + 1) * D]
                sq = p_sq.tile([P, D], f32, tag="sq", name="sq")
                nc.vector.tensor_tensor_reduce(
                    out=sq[:], in0=a, in1=a,
                    scale=1.0, scalar=0.0,
                    op0=Op.mult, op1=Op.add,
                    accum_out=ss[:, t0 + t:t0 + t + 1],
                )

            # ---- 1/norm = exp(-0.5 * ln(sumsq)); single ACT table set ----
            lns = p_sq.tile([P, spt], f32, tag="lns", name="lns")
            nc.scalar.activation(lns[:], ss[:, t0:t0 + spt], Fn.Ln)
            nc.scalar.activation(
                inv[:, t0:t0 + spt], lns[:], Fn.Exp, scale=-0.5)

            # ---- normalize slice to bf16 in ONE DVE op (broadcast inv) ----
            inv_bc = inv[:, t0:t0 + spt].rearrange(
                "p (t o) -> p t o", o=1).broadcast_to((P, spt, D))
            nc.vector.tensor_mul(
                zn[g][:, l0 * D:(l0 + spt) * D].rearrange(
                    "p (t c) -> p t c", c=D),
                znat[g][:, l0 * D:(l0 + spt) * D].rearrange(
                    "p (t c) -> p t c", c=D),
                inv_bc)

            # ---- bounce bf16 to DRAM (Pool), then 2 transposed xbar
            # ---- loads [rows,128]->[128,rows] (SP), then fp8 cast (Pool) ---
            nc.gpsimd.dma_start(
                out=zbounce[g][l0 * P:(l0 + spt) * P, :].rearrange(
                    "(t p) c -> p t c", p=P),
                in_=zn[g][:, l0 * D:(l0 + spt) * D].rearrange(
                    "p (t c) -> p t c", c=D),
            )
            for k in range(2):
                nc.sync.dma_start_transpose(
                    zntb[g][:, k, l0 * P:(l0 + spt) * P],
                    zbounce[g][l0 * P:(l0 + spt) * P, k * P:(k + 1) * P],
                )
            nc.gpsimd.tensor_copy(
                out=znt[g][:, :, l0 * P:(l0 + spt) * P],
                in_=zntb[g][:, :, l0 * P:(l0 + spt) * P])

        # ---- raw positive dots: own tiles (g0) x partner tiles (g4) ----
        if g == 4:
            for i in range(NI):
                sq = p_sq.tile([P, D], f32, tag="sq", name="sqp")
                nc.vector.tensor_tensor_reduce(
                    out=sq[:],
                    in0=znat[0][:, i * D:(i + 1) * D],
                    in1=znat[4][:, i * D:(i + 1) * D],
                    scale=1.0, scalar=0.0,
                    op0=Op.mult, op1=Op.add,
                    accum_out=posr[:, i:i + 1],
                )

    # All preps are emitted up front: the tile scheduler is ready-driven
    # with emission-order priority, so each group's small ACT/DVE ops
    # preempt the long exp stream the moment their inputs land, while
    # the exps (lower priority, but ready) fill the gaps.
    prep(0, nsl=4)
    prep(1, nsl=2)
    for g in range(2, NGRP):
        prep(g)

    # ---- main loop: column-major S row-blocks + fused exp/rowsum ----
    for cc in range(NCC):
        for t in range(NI):
            ps = p_ps.tile([P, CC], f32, tag="ps", name="psmm")
            for jc in range(CC // CH):
                col = cc * CC + jc * CH
                g, c0 = divmod(col, TPG * P)
                nc.tensor.matmul(
                    ps[:, jc * CH:(jc + 1) * CH],
                    lhsT=znt[0][:, :, t * P:(t + 1) * P],
                    rhs=znt[g][:, :, c0:c0 + CH],
                    start=True, stop=True,
                    perf_mode=mybir.MatmulPerfMode.DoubleRow,
                )
            ex = p_ex.tile([P, CC], fp8, tag="ex", name="ex")
            nc.scalar.activation(
                ex[:], ps[:], Fn.Exp, scale=2.0,
                accum_out=sums[:, t * NCC + cc:t * NCC + cc + 1],
            )

    # ---- loss assembly: loss = -2*pos + ln(rowsum - exp(2*selfsim)) ----
    totals = p_stats.tile([P, NI], f32, tag="tot")
    nc.vector.tensor_reduce(
        totals[:], sums[:].rearrange("p (i c) -> p i c", c=NCC),
        axis=mybir.AxisListType.X, op=Op.add,
    )
    s1 = p_stats.tile([P, NI], f32, tag="s1")
    nc.vector.tensor_tensor(s1[:], ss[:, 0:NI], inv[:, 0:NI], op=Op.mult)
    s2 = p_stats.tile([P, NI], f32, tag="s2")
    nc.vector.tensor_tensor(s2[:], s1[:], inv[:, 0:NI], op=Op.mult)
    es = p_stats.tile([P, NI], f32, tag="es")
    nc.scalar.activation(es[:], s2[:], Fn.Exp, scale=2.0)
    neg = p_stats.tile([P, NI], f32, tag="neg")
    nc.vector.tensor_sub(neg[:], totals[:], es[:])
    lg = p_stats.tile([P, NI], f32, tag="lg")
    nc.scalar.activation(lg[:], neg[:], Fn.Ln)
    p1 = p_stats.tile([P, NI], f32, tag="p1")
    nc.vector.tensor_tensor(p1[:], posr[:], inv[:, 0:NI], op=Op.mult)
    p2 = p_stats.tile([P, NI], f32, tag="p2")
    nc.vector.tensor_tensor(
        p2[:], p1[:], inv[:, NT // 2:NT // 2 + NI], op=Op.mult)
    loss = p_stats.tile([P, NI], f32, tag="loss")
    nc.vector.scalar_tensor_tensor(
        out=loss[:], in0=p2[:], scalar=-2.0 / (2.0 * TAU), in1=lg[:],
        op0=Op.mult, op1=Op.add,
    )
    nc.sync.dma_start(out=out_ap, in_=loss[:])


def build_nc():
    """Build (once) the Bass module shared by all 8 cores."""
    from contextlib import ExitStack

    from concourse import bacc, mybir
    import concourse.tile as tile

    nc = bacc.Bacc("TRN2", target_bir_lowering=False, debug=False)
    zr = nc.dram_tensor("zr", [N, D], mybir.dt.float32,
                        kind="ExternalInput").ap()
    out = nc.dram_tensor("out", [P, NI], mybir.dt.float32,
                         kind="ExternalOutput").ap()
    with tile.TileContext(nc) as tc:
        with ExitStack() as ctx:
            _kernel_body(ctx, tc, out, zr)
    return nc


_NC = None


def _get_nc(finalized=True):
    global _NC
    if _NC is None:
        _NC = build_nc()
    if finalized and not _NC.is_finalized():
        _NC.finalize()
    return _NC


def make_in_maps(z_orig, z_augment):
    z = np.ascontiguousarray(
        np.concatenate([np.asarray(z_augment, dtype=np.float32),
                        np.asarray(z_orig, dtype=np.float32)], axis=0))
    return [{"zr": np.roll(z, -ROWS_PER_CORE * c, axis=0)}
            for c in range(N_CORES)]


def reduce_outputs(results):
    total = 0.0
    for r in results:
        total += float(np.asarray(r["out"], dtype=np.float64).sum())
    return np.float32(total / N)


def kernel(z_orig, z_augment):
    from concourse.bass_utils import run_bass_kernel_spmd

    nc = _get_nc()
    in_maps = make_in_maps(z_orig, z_augment)
    res = run_bass_kernel_spmd(nc, in_maps, core_ids=list(range(N_CORES)))
    return reduce_outputs(res.results)


# revision 7
# speedup vs baseline: 1.3091x; 1.0775x over previous
"""Trainium2 Bass kernel for nn_ContrastiveLoss2 (SimCLR-style NT-Xent loss).

Math (matches the jax reference):
    z  = concat([z_augment, z_orig])                       # [N=8192, D=256]
    zn = z / max(||z||, eps)                               # row L2 normalize
    S  = zn @ zn.T                                         # cosine sim [N, N]
    loss_i = -S[i, i+-B]/tau + log( sum_{j != i} exp(S[i,j]/tau) )
    out = mean_i loss_i                                    # tau = 0.5

Key identity: the softmax denominator is the full row sum of exp(S/tau)
minus the diagonal term exp(S_ii/tau).

Distribution: data-parallel over the 8192 rows -> 1024 rows per core.
Each core receives the full z ROTATED so that its own rows sit at
[0:1024) and the positive partners at [4096:5120).  Pure SPMD, no
collectives; the host sums the 8 per-core partial losses.

Per-core pipeline (engine assignment in parentheses):
  - load z in 8 groups of 1024 rows (SP DMA)
  - per-tile sum-of-squares via fused tensor_tensor_reduce (DVE)
  - 1/norm = exp(-0.5*ln(sumsq)) (ACT, one table set)
  - zn = z * invnorm -> bf16, one fused DVE op per group slice
  - bf16 bounce to DRAM (Pool DMA) + transposed reload (SP xbar DMA)
  - bf16 -> fp8e4 cast of the transposed operand (Pool)
  - S row-blocks via fp8 DoubleRow matmul (PE), K=256 in one pass
  - exp(2*S) + row-sum via activation accum_out (ACT), 2048-col chunks,
    column-major over the sim matrix so the ACT queue never starves
  - loss assembly (DVE/ACT) -> [128, 8] per-row losses -> DRAM

Groups 0/1 are prepped in fine 256-row slices to shorten the pipeline
ramp before the first exp; later groups are prepped 1024 rows at a
time, emitted between exp column-blocks so every engine queue stays
in dependency order.
"""

import sys

import numpy as np

try:
    import concourse  # noqa: F401
except ImportError:  # pragma: no cover
    sys.path.insert(0, "/opt/trn_rl_repo")

N_CORES = 8
N = 8192          # total rows (2B)
D = 256           # feature dim
B = 4096          # batch (positive offset)
ROWS_PER_CORE = N // N_CORES   # 1024
P = 128           # SBUF partitions
NT = N // P       # 64 natural row-tiles
NGRP = 8          # prep groups (1024 rows each)
TPG = NT // NGRP  # 8 tiles per group
NI = ROWS_PER_CORE // P        # 8 own row-tiles
CH = 512          # matmul chunk (one PSUM bank of fp32)
CC = 2048         # ACT exp chunk width = 4 PSUM banks
NCC = N // CC     # 4 column chunks
TAU = 0.5


def _kernel_body(ctx, tc, out_ap, zr_ap):
    import concourse.bass as bass  # noqa: F401
    from concourse import mybir

    nc = tc.nc
    f32 = mybir.dt.float32
    bf16 = mybir.dt.bfloat16
    fp8 = mybir.dt.float8e4
    Fn = mybir.ActivationFunctionType
    Op = mybir.AluOpType

    p_znat = ctx.enter_context(tc.tile_pool(name="znat", bufs=1))
    p_zn = ctx.enter_context(tc.tile_pool(name="zn", bufs=1))
    p_zntb = ctx.enter_context(tc.tile_pool(name="zntb", bufs=1))
    p_stats = ctx.enter_context(tc.tile_pool(name="stats", bufs=1))
    p_sq = ctx.enter_context(tc.tile_pool(name="sq", bufs=2))
    p_ex = ctx.enter_context(tc.tile_pool(name="ex", bufs=2))
    p_ps = ctx.enter_context(tc.tile_pool(name="ps", bufs=2, space="PSUM"))
    p_dram = ctx.enter_context(tc.tile_pool(name="dram", bufs=1, space="DRAM"))

    # per-group tiles: keeps the dependency tracker's byte ranges disjoint
    # so transposes/casts of group g never serialize behind group g+1
    znat = [p_znat.tile([P, TPG * D], f32, tag=f"znat{g}", name=f"znat{g}")
            for g in range(NGRP)]
    zn = [p_zn.tile([P, TPG * D], bf16, tag=f"zn{g}", name=f"zn{g}")
          for g in range(NGRP)]
    zntb = [p_zntb.tile([P, 2, TPG * P], bf16, tag=f"zntb{g}", name=f"zntb{g}")
            for g in range(NGRP)]
    zbounce = [p_dram.tile([TPG * P, D], bf16, tag=f"zb{g}", name=f"zb{g}")
               for g in range(NGRP)]
    ss = p_stats.tile([P, NT], f32, tag="ss")     # per-row sum of squares
    inv = p_stats.tile([P, NT], f32, tag="inv")   # per-row 1/norm
    posr = p_stats.tile([P, NI], f32, tag="posr")  # raw dot(z_i, z_partner)
    sums = p_stats.tile([P, NI * NCC], f32, tag="sums")  # exp row-sum parts

    def prep(g, nsl=1):
        """Prepare group g (1024 rows) in nsl pipeline slices."""
        spt = TPG // nsl          # tiles per slice
        for s in range(nsl):
            t0 = g * TPG + s * spt        # global first tile of slice
            l0 = s * spt                  # tile offset within group tiles
            # ---- load rotated rows [t0*128, (t0+spt)*128) (SP) ----
            nc.sync.dma_start(
                out=znat[g][:, l0 * D:(l0 + spt) * D].rearrange(
                    "p (t c) -> p t c", c=D),
                in_=zr_ap[t0 * P:(t0 + spt) * P, :].rearrange(
                    "(t p) c -> p t c", p=P),
            )

            # ---- per-tile sumsq via fused mul+reduce (DVE) ----
            for t in range(spt):
                a = znat[g][:, (l0 + t) * D:(l0 + t + 1) * D]
                sq = p_sq.tile([P, D], f32, tag="sq", name="sq")
                nc.vector.tensor_tensor_reduce(
                    out=sq[:], in0=a, in1=a,
                    scale=1.0, scalar=0.0,
                    op0=Op.mult, op1=Op.add,
                    accum_out=ss[:, t0 + t:t0 + t + 1],
                )

            # ---- 1/norm = exp(-0.5 * ln(sumsq)); single ACT table set ----
            lns = p_sq.tile([P, spt], f32, tag="lns", name="lns")
            nc.scalar.activation(lns[:], ss[:, t0:t0 + spt], Fn.Ln)
            nc.scalar.activation(
                inv[:, t0:t0 + spt], lns[:], Fn.Exp, scale=-0.5)

            # ---- normalize slice to bf16 in ONE DVE op (broadcast inv) ----
            inv_bc = inv[:, t0:t0 + spt].rearrange(
                "p (t o) -> p t o", o=1).broadcast_to((P, spt, D))
            nc.vector.tensor_mul(
                zn[g][:, l0 * D:(l0 + spt) * D].rearrange(
                    "p (t c) -> p t c", c=D),
                znat[g][:, l0 * D:(l0 + spt) * D].rearrange(
                    "p (t c) -> p t c", c=D),
                inv_bc)

            # ---- bounce bf16 to DRAM (Pool), then 2 transposed xbar
            # ---- loads [rows,128]->[128,rows] (SP), then fp8 cast (Pool) ---
            nc.gpsimd.dma_start(
                out=zbounce[g][l0 * P:(l0 + spt) * P, :].rearrange(
                    "(t p) c -> p t c", p=P),
                in_=zn[g][:, l0 * D:(l0 + spt) * D].rearrange(
                    "p (t c) -> p t c", c=D),
            )
            for k in range(2):
                nc.sync.dma_start_transpose(
                    zntb[g][:, k, l0 * P:(l0 + spt) * P],
                    zbounce[g][l0 * P:(l0 + spt) * P, k * P:(k + 1) * P],
                )

        # ---- raw positive dots: own tiles (g0) x partner tiles (g4) ----
        if g == 4:
            for i in range(NI):
                sq = p_sq.tile([P, D], f32, tag="sq", name="sqp")
                nc.vector.tensor_tensor_reduce(
                    out=sq[:],
                    in0=znat[0][:, i * D:(i + 1) * D],
                    in1=znat[4][:, i * D:(i + 1) * D],
                    scale=1.0, scalar=0.0,
                    op0=Op.mult, op1=Op.add,
                    accum_out=posr[:, i:i + 1],
                )

    # Warm the ACT Ln/Exp table at t=0 so the first real Ln doesn't pay
    # the 1.3us table load on the critical path.
    warm = p_stats.tile([P, 1], f32, tag="warm")
    nc.vector.memset(warm[:], 1.0)
    wrm2 = p_stats.tile([P, 1], f32, tag="warm2")
    nc.scalar.activation(wrm2[:], warm[:], Fn.Ln)
    nc.scalar.activation(wrm2[:], warm[:], Fn.Exp)

    # All preps are emitted up front: the tile scheduler is ready-driven
    # with emission-order priority, so each group's small ACT/DVE ops
    # preempt the long exp stream the moment their inputs land, while
    # the exps (lower priority, but ready) fill the gaps.
    prep(0, nsl=4)
    prep(1, nsl=2)
    for g in range(2, NGRP):
        prep(g)

    # ---- main loop: column-major S row-blocks + fused exp/rowsum ----
    for cc in range(NCC):
        for t in range(NI):
            ps = p_ps.tile([P, CC], f32, tag="ps", name="psmm")
            for jc in range(CC // CH):
                col = cc * CC + jc * CH
                g, c0 = divmod(col, TPG * P)
                for k in range(2):
                    nc.tensor.matmul(
                        ps[:, jc * CH:(jc + 1) * CH],
                        lhsT=zntb[0][:, k, t * P:(t + 1) * P],
                        rhs=zntb[g][:, k, c0:c0 + CH],
                        start=(k == 0), stop=(k == 1),
                    )
            ex = p_ex.tile([P, CC], fp8, tag="ex", name="ex")
            nc.scalar.activation(
                ex[:], ps[:], Fn.Exp, scale=2.0,
                accum_out=sums[:, t * NCC + cc:t * NCC + cc + 1],
            )

    # ---- loss assembly: loss = -2*pos + ln(rowsum - exp(2*selfsim)) ----
    totals = p_stats.tile([P, NI], f32, tag="tot")
    nc.vector.tensor_reduce(
        totals[:], sums[:].rearrange("p (i c) -> p i c", c=NCC),
        axis=mybir.AxisListType.X, op=Op.add,
    )
    s1 = p_stats.tile([P, NI], f32, tag="s1")
    nc.vector.tensor_tensor(s1[:], ss[:, 0:NI], inv[:, 0:NI], op=Op.mult)
    s2 = p_stats.tile([P, NI], f32, tag="s2")
    nc.vector.tensor_tensor(s2[:], s1[:], inv[:, 0:NI], op=Op.mult)
    es = p_stats.tile([P, NI], f32, tag="es")
    nc.scalar.activation(es[:], s2[:], Fn.Exp, scale=2.0)
    neg = p_stats.tile([P, NI], f32, tag="neg")
    nc.vector.tensor_sub(neg[:], totals[:], es[:])
    lg = p_stats.tile([P, NI], f32, tag="lg")
    nc.scalar.activation(lg[:], neg[:], Fn.Ln)
    p1 = p_stats.tile([P, NI], f32, tag="p1")
    nc.vector.tensor_tensor(p1[:], posr[:], inv[:, 0:NI], op=Op.mult)
    p2 = p_stats.tile([P, NI], f32, tag="p2")
    nc.vector.tensor_tensor(
        p2[:], p1[:], inv[:, NT // 2:NT // 2 + NI], op=Op.mult)
    loss = p_stats.tile([P, NI], f32, tag="loss")
    nc.vector.scalar_tensor_tensor(
        out=loss[:], in0=p2[:], scalar=-2.0 / (2.0 * TAU), in1=lg[:],
        op0=Op.mult, op1=Op.add,
    )
    nc.sync.dma_start(out=out_ap, in_=loss[:])


def build_nc():
    """Build (once) the Bass module shared by all 8 cores."""
    from contextlib import ExitStack

    from concourse import bacc, mybir
    import concourse.tile as tile

    nc = bacc.Bacc("TRN2", target_bir_lowering=False, debug=False)
    zr = nc.dram_tensor("zr", [N, D], mybir.dt.float32,
                        kind="ExternalInput").ap()
    out = nc.dram_tensor("out", [P, NI], mybir.dt.float32,
                         kind="ExternalOutput").ap()
    with tile.TileContext(nc) as tc:
        with ExitStack() as ctx:
            _kernel_body(ctx, tc, out, zr)
    return nc


_NC = None


def _get_nc(finalized=True):
    global _NC
    if _NC is None:
        _NC = build_nc()
    if finalized and not _NC.is_finalized():
        _NC.finalize()
    return _NC


def make_in_maps(z_orig, z_augment):
    z = np.ascontiguousarray(
        np.concatenate([np.asarray(z_augment, dtype=np.float32),
                        np.asarray(z_orig, dtype=np.float32)], axis=0))
    return [{"zr": np.roll(z, -ROWS_PER_CORE * c, axis=0)}
            for c in range(N_CORES)]


def reduce_outputs(results):
    total = 0.0
    for r in results:
        total += float(np.asarray(r["out"], dtype=np.float64).sum())
    return np.float32(total / N)


def kernel(z_orig, z_augment):
    from concourse.bass_utils import run_bass_kernel_spmd

    nc = _get_nc()
    in_maps = make_in_maps(z_orig, z_augment)
    res = run_bass_kernel_spmd(nc, in_maps, core_ids=list(range(N_CORES)))
    return reduce_outputs(res.results)
